# revision 11
# baseline (speedup 1.0000x reference)
"""GCNConv Trainium2 kernel: out = segment_sum(w_e * (x @ W)[src_e] -> dst_e) + bias.

Distribution (8-core SPMD, one program):
  - Destination nodes sharded across 8 cores (rows of the output).
  - Aggregation runs in x-space (in_dim features), transformed by W once per
    128-dst window at the end: out = (sum_e w_e x[src_e]) @ W + bias.

Per core:
  - Host sorts that core's edges into a "tape" of 128-edge slots:
    bank-major (src // 32768, so dma_gather's int16 indices reach), then by
    128-dst window; each (bank, window) run padded to whole 128-slot blocks,
    with a block count uniform across cores (SPMD requires one program).
  - Device: big dma_gather instructions pull x[src] rows (512B, line rate)
    for 4096 tape slots at a time, landing as [128, 32 blocks, 128] tiles.
    Per block: one DVE tensor_scalar builds S[slot, d] = (colidx==dstoff)*w,
    one PE matmul accumulates aggT += Xg.T @ S into a PSUM window tile.
    Run end: DVE adds PSUM into the SBUF accumulator agg[128 feat, nwin*128].
  - Final: per window, PE matmul agg_w.T @ W -> PSUM, DVE adds bias, store.
"""

import sys

sys.path.insert(0, "/opt/trn_rl_repo")

import ml_dtypes
import numpy as np

from concourse import bacc, bass, mybir, tile
from concourse.bass_utils import run_bass_kernel_spmd

N_CORES = 8
P = 128  # partitions / block size / dst window size
BANK = 32768  # src rows reachable by one gather (int16 indices)
GBIG = 4096  # tape slots per dma_gather instruction
SG = 4  # blocks per S-build group


def _preprocess(n_nodes, edge_index, edge_weight):
    """Build per-core tapes. Returns dict of host arrays + block structure."""
    n_per_core = n_nodes // N_CORES
    assert n_per_core * N_CORES == n_nodes
    nwin = -(-n_per_core // P)
    nbank = -(-n_nodes // BANK)

    dst = edge_index[0].astype(np.int64)
    src = edge_index[1].astype(np.int64)
    w = edge_weight.astype(np.float32)
    E = dst.shape[0]

    core = dst // n_per_core
    loc = dst - core * n_per_core
    win = loc // P
    off = (loc - win * P).astype(np.float32)
    bank = src // BANK
    src_local = (src - bank * BANK).astype(np.int16)

    nrun = nbank * nwin  # runs per core, bank-major
    key = (core * nbank + bank) * nwin + win
    order = np.argsort(key, kind="stable")
    skey = key[order]

    cnt = np.bincount(key, minlength=N_CORES * nrun).reshape(N_CORES, nrun)
    blocks_per_run = -(-cnt.max(axis=0) // P)  # uniform across cores; may be 0
    B = int(blocks_per_run.sum())
    cumb = np.concatenate([[0], np.cumsum(blocks_per_run)])

    # slot position of each edge within its core's tape
    starts = np.r_[0, np.flatnonzero(np.diff(skey)) + 1]
    run_len = np.diff(np.r_[starts, E])
    run_id = np.repeat(np.arange(len(starts)), run_len)
    pos_in_run = np.arange(E) - starts[run_id]
    slot = cumb[skey % nrun] * P + pos_in_run

    src_arr = np.zeros((N_CORES, B * P), np.int16)
    off_arr = np.zeros((N_CORES, B * P), np.float32)
    w_arr = np.zeros((N_CORES, B * P), np.float32)
    flat = (skey // nrun) * (B * P) + slot
    src_arr.reshape(-1)[flat] = src_local[order]
    off_arr.reshape(-1)[flat] = off[order]
    w_arr.reshape(-1)[flat] = w[order]

    # idx tape wrapped in 16 partitions, replicated 8x: idx[16g+p, s] = tape[16s+p]
    idxw = src_arr.reshape(N_CORES, B * P // 16, 16).transpose(0, 2, 1)
    idx_np = np.tile(idxw, (1, 8, 1)).copy()  # [C, 128, B*P//16]

    # precomputed S rows, partition-major: S_host[c, p, b*P + dstoff] = w
    # (one 128-wide scaled one-hot per tape slot, streamed contiguously)
    s_host = np.zeros((N_CORES, P, B * P), ml_dtypes.bfloat16)
    core_s = skey // nrun
    blk = slot // P
    lane = slot - blk * P
    s_host[core_s, lane, blk * P + off[order].astype(np.int64)] = w[order].astype(
        ml_dtypes.bfloat16
    )

    run_of_block = np.repeat(np.arange(nrun), blocks_per_run)
    return dict(
        idx=idx_np,
        s_host=s_host,
        B=B,
        nwin=nwin,
        nbank=nbank,
        n_per_core=n_per_core,
        run_of_block=run_of_block,
        blocks_per_run=blocks_per_run,
    )


def _build_program(n_nodes, in_dim, out_dim, pp):
    B, nwin, nbank = pp["B"], pp["nwin"], pp["nbank"]
    run_of_block = pp["run_of_block"]
    blocks_per_run = pp["blocks_per_run"]

    nc = bacc.Bacc(
        "TRN2",
        target_bir_lowering=False,
        debug=False,
        num_devices=N_CORES,
        num_swdge_queues=4,
    )
    f32 = mybir.dt.float32
    bf16 = mybir.dt.bfloat16
    i16 = mybir.dt.int16

    x_d = nc.declare_dram_parameter("xbf", [n_nodes, in_dim], bf16, isOutput=False)
    idx_d = nc.declare_dram_parameter("idx", [P, B * P // 16], i16, isOutput=False)
    smat_d = nc.declare_dram_parameter("smat", [P, B * P], bf16, isOutput=False)
    wmat_d = nc.declare_dram_parameter("wmat", [in_dim, out_dim], f32, isOutput=False)
    bias_d = nc.declare_dram_parameter("biasrep", [P, out_dim], f32, isOutput=False)
    out_d = nc.declare_dram_parameter("out", [nwin * P, out_dim], f32, isOutput=True)

    first = np.r_[True, run_of_block[1:] != run_of_block[:-1]]
    last = np.r_[first[1:], True]
    # block index whose completion finishes each window (last nonempty run)
    final_blk_of_win = {}
    for b in range(B):
        if last[b]:
            final_blk_of_win[int(run_of_block[b]) % nwin] = b
    finals_at = {v: k for k, v in final_blk_of_win.items()}

    # gather schedule: chop each bank's tape segment into GBIG-slot chunks
    # (chunks are block-aligned; blocks never span banks)
    bank_of_block = run_of_block // nwin
    gathers = []  # (block_start, n_blocks, bank)
    b0 = 0
    while b0 < B:
        k = bank_of_block[b0]
        b1 = b0
        while b1 < B and bank_of_block[b1] == k and (b1 - b0) * P < GBIG:
            b1 += 1
        gathers.append((b0, b1 - b0, int(k)))
        b0 = b1

    with tile.TileContext(nc) as tc:
        with (
            tc.tile_pool(name="const", bufs=1) as const_tp,
            tc.tile_pool(name="meta", bufs=1) as meta_tp,
            tc.tile_pool(name="agg", bufs=1) as agg_tp,
            tc.tile_pool(name="g", bufs=6) as g_tp,
            tc.tile_pool(name="s", bufs=3) as s_tp,
            tc.tile_pool(name="outsb", bufs=3) as outsb_tp,
            tc.tile_pool(name="psum_agg", bufs=4, space="PSUM") as psum_agg_tp,
            tc.tile_pool(name="psum_out", bufs=3, space="PSUM") as psum_out_tp,
        ):
            wmat_t = const_tp.tile([in_dim, out_dim], f32)
            nc.sync.dma_start(out=wmat_t[:], in_=wmat_d[:, :])
            bias_t = const_tp.tile([P, out_dim], f32)
            nc.sync.dma_start(out=bias_t[:], in_=bias_d[:, :])

            idx_t = meta_tp.tile([P, B * P // 16], i16)
            nc.sync.dma_start(out=idx_t[:], in_=idx_d[:, :])
            agg_t = agg_tp.tile([in_dim, nwin * P], f32)
            nc.vector.memset(agg_t[:], 0.0)

            # aggregation: walk gathers; inner loop over their blocks
            aggT_psum = None
            for gi, (g0, gnb, k) in enumerate(gathers):
                n_idx = gnb * P
                g_t = g_tp.tile([P, gnb * in_dim], bf16, tag="g")
                nc.gpsimd.dma_gather(
                    out_ap=g_t[:].rearrange("p (c e) -> p c e", e=in_dim),
                    in_ap=x_d[k * BANK :, :],
                    idxs_ap=idx_t[:, g0 * P // 16 : (g0 + gnb) * P // 16],
                    num_idxs=n_idx,
                    num_idxs_reg=n_idx,
                    elem_size=in_dim,
                    single_packet=False,
                    queue_num=gi % 4,
                )
                s_t = s_tp.tile([P, GBIG], bf16, tag="s")
                nc.sync.dma_start(
                    out=s_t[:, : gnb * P],
                    in_=smat_d[:, g0 * P : (g0 + gnb) * P],
                )
                for j in range(gnb):
                    b = g0 + j
                    if first[b]:
                        aggT_psum = psum_agg_tp.tile([in_dim, P], f32, tag="aggT")
                    nc.tensor.matmul(
                        out=aggT_psum[:],
                        lhsT=g_t[:, j * in_dim : (j + 1) * in_dim],
                        rhs=s_t[:, j * P : (j + 1) * P],
                        start=bool(first[b]),
                        stop=bool(last[b]),
                    )
                    if last[b]:
                        r = run_of_block[b]
                        w_i = r % nwin
                        nc.vector.tensor_add(
                            out=agg_t[:, w_i * P : (w_i + 1) * P],
                            in0=agg_t[:, w_i * P : (w_i + 1) * P],
                            in1=aggT_psum[:],
                        )
                        if finals_at.get(b) is not None:
                            # window complete: transform + store now
                            out_psum = psum_out_tp.tile(
                                [P, out_dim], f32, tag="out_psum"
                            )
                            nc.tensor.matmul(
                                out=out_psum[:],
                                lhsT=agg_t[:, w_i * P : (w_i + 1) * P],
                                rhs=wmat_t[:],
                                start=True,
                                stop=True,
                            )
                            out_sb = outsb_tp.tile([P, out_dim], f32, tag="out_sb")
                            nc.vector.tensor_add(
                                out=out_sb[:], in0=out_psum[:], in1=bias_t[:]
                            )
                            nc.sync.dma_start(
                                out=out_d[w_i * P : (w_i + 1) * P, :], in_=out_sb[:]
                            )

            # windows with no edges anywhere: out = bias
            for w_i in range(nwin):
                if w_i not in final_blk_of_win:
                    out_sb = outsb_tp.tile([P, out_dim], f32, tag="out_sb")
                    nc.vector.tensor_copy(out=out_sb[:], in_=bias_t[:])
                    nc.sync.dma_start(
                        out=out_d[w_i * P : (w_i + 1) * P, :], in_=out_sb[:]
                    )

    nc.compile()
    return nc


def kernel(x, edge_index, edge_weight, weight, bias):
    x = np.asarray(x, np.float32)
    edge_index = np.asarray(edge_index, np.int32)
    edge_weight = np.asarray(edge_weight, np.float32)
    weight = np.asarray(weight, np.float32)
    bias = np.asarray(bias, np.float32)

    n_nodes, in_dim = x.shape
    out_dim = weight.shape[1]

    pp = _preprocess(n_nodes, edge_index, edge_weight)
    nc = _build_program(n_nodes, in_dim, out_dim, pp)

    biasrep = np.broadcast_to(bias, (P, out_dim)).copy()
    xbf = x.astype(ml_dtypes.bfloat16)
    in_maps = [
        {
            "xbf": xbf,
            "idx": pp["idx"][c],
            "smat": pp["s_host"][c].reshape(P, -1),
            "wmat": weight,
            "biasrep": biasrep,
        }
        for c in range(N_CORES)
    ]

    res = run_bass_kernel_spmd(nc, in_maps, core_ids=list(range(N_CORES)))
    npc = pp["n_per_core"]
    out = np.concatenate(
        [res.results[c]["out"][:npc] for c in range(N_CORES)], axis=0
    )
    return out.astype(np.float32)


if __name__ == "__main__":
    rng = np.random.default_rng(0)
    N, E, DI, DO = 1024, 4096, 128, 64
    if len(sys.argv) > 1 and sys.argv[1] == "big":
        N, E = 100000, 1600000
    x = rng.standard_normal((N, DI), dtype=np.float32)
    ei = rng.integers(0, N, (2, E)).astype(np.int32)
    ew = rng.random(E, dtype=np.float32)
    wm = rng.standard_normal((DI, DO), dtype=np.float32) * 0.125
    bs = rng.standard_normal(DO, dtype=np.float32)

    out = kernel(x, ei, ew, wm, bs)

    h = x @ wm
    ref = np.zeros((N, DO), np.float32)
    np.add.at(ref, ei[0], ew[:, None] * h[ei[1]])
    ref += bs
    err = np.abs(out - ref).max() / (np.abs(ref).max() + 1e-9)
    print("max rel err:", err)
